# revision 8
# baseline (speedup 1.0000x reference)
"""3-layer GAT on 8 Trainium2 NeuronCores (Bass/Tile).

Sharding: dst-node data parallel. Nodes are split 1250/core (padded 1280).
Per layer: a dense phase computes h = x @ W_aug (W_aug carries extra columns
producing e_src/e_dst attention logits). Per-node rows are packed fp8:
[1024B fp8 h | 8B fp16 e_src | pad] (1280B, L1/L2) or [726B fp8 h | pad |
12B fp16 e_src | pad] (768B, L3) — stored as fp16 tensors, fp8 accessed via
bitcast views. Layer 1 computes the table redundantly on every core; layers
2/3 all-gather it tile-pair-by-pair so the collective overlaps the previous
layer's edge phase. The edge phase gathers per-edge rows with one dma_gather
per dst-tile (desc-emission-bound at ~8.7ns/row, so bytes are nearly free;
trailing pad slots use idx=-1 + a runtime count register so their descriptors
are skipped), computes exp(leaky_relu(e_src+e_dst) - shift) on ACT/DVE,
applies the per-head alpha scale fused with the fp8->fp16 upcast into a
separate gs tile, and aggregates per 128-dst tile with one-hot
[128e x 128d] matmuls accumulated in PSUM (denominator in a parallel psC).
Epilogues normalize, apply elu/residual/sigmoid, and transpose the
activations (PE transpose) for the next dense phase.

Feature columns are interleaved (c*H + h) so per-head scaling is a single
stride-0-broadcast DVE multiply; weights are permuted accordingly on host.
Matmuls run fp16 with fp32 PSUM accumulation; only the gathered h payload is
fp8 (validated ~5e-3 max rel err vs fp32 reference, gate 2e-2).
"""
import sys

sys.path.insert(0, "/opt/trn_rl_repo")

import numpy as np

NCORES, N, NPC, NPAD, T, P = 8, 10000, 1250, 1280, 10, 128
R = 10240            # table rows (tile-major: TROW = t*1024 + q*128 + p)
SHIFTS = (3.5, 1.25, 1.0)
# fp16-typed table widths (bytes/2): L1/L2 row = 1024 fp8 h + 4 fp16 e_src
# + pad -> 640 fp16 cols; L3 row = 726 fp8 h + pad + 6 fp16 e_src -> 384.
WT12, WT3 = 640, 384
ESOFF12, ESOFF3 = 512, 364   # fp16 col offset of e_src in a table row

F16 = np.float16


def _trow(g):
    # Row layout matches the 2-tile AllGather concat: [tile-pair][rank][tile][p]
    q, r = g // NPC, g % NPC
    t, p = r // P, r % P
    return (t // 2) * 2048 + q * 256 + (t % 2) * P + p


def _wrap_idx(idx):
    """[n] -> [128, n//16] int16 (wrapped in 16 partitions, replicated 8x)."""
    blk = idx.astype(np.int16).reshape(-1, 16).T.copy()
    return np.tile(blk, (8, 1))


def preprocess(inputs):
    x = np.asarray(inputs["x"], np.float32)
    ei = np.asarray(inputs["edge_index"])
    src = np.concatenate([ei[0], np.arange(N)]).astype(np.int64)
    dst = np.concatenate([ei[1], np.arange(N)]).astype(np.int64)
    order = np.argsort(dst, kind="stable")
    src, dst = src[order], dst[order]

    # per-(core,tile) edge lists and uniform chunk grid
    per = []
    K_T = 0
    for c in range(NCORES):
        m = (dst >= c * NPC) & (dst < (c + 1) * NPC)
        s, d = src[m], dst[m] - c * NPC
        tiles = []
        for t in range(T):
            mt = (d >= t * P) & (d < (t + 1) * P)
            tiles.append((s[mt], d[mt] - t * P))
            K_T = max(K_T, (int(mt.sum()) + P - 1) // P)
        per.append(tiles)
    NCH = T * K_T

    gidx, gidxA, sts, st2s, xlocs, cnts = [], [], [], [], [], []
    for c in range(NCORES):
        ss = np.zeros((T, K_T * P), np.int64)
        dd = np.zeros((T, K_T * P), np.int64)
        vv = np.zeros((T, K_T * P), bool)
        cnt = np.zeros(T, np.int32)
        for t in range(T):
            s, d = per[c][t]
            n = len(s)
            ss[t, :n], dd[t, :n], vv[t, :n] = s, d, True
            cnt[t] = n
        rows = _trow(ss)
        rows_neg = np.where(vv, rows, -1)
        gidx.append(_wrap_idx(rows_neg.reshape(-1)))
        # L1 tiles 0/1: full gathers (pad = row 0) so the two g buffers are
        # initialized with finite values before negative-skip gathers reuse
        # their stale contents for pad slots.
        rows_full = np.where(vv, rows, 0)
        gidxA.append(_wrap_idx(rows_full[:2].reshape(-1)))
        cnts.append(np.tile(cnt, (128, 1)))
        # S (edge->dst one-hot) and S2 (dst->edge) per chunk
        S = np.zeros((T, K_T * P, P), F16)
        ar = np.arange(K_T * P)
        for t in range(T):
            sl = ar[vv[t]]
            S[t, sl, dd[t][vv[t]]] = 1.0
        S = S.reshape(NCH, P, P)
        sts.append(np.ascontiguousarray(
            S.transpose(1, 0, 2).reshape(P, NCH * P)))   # [edge%128, chunk*128+dst]
        st2s.append(np.ascontiguousarray(
            S.transpose(2, 0, 1).reshape(P, NCH * P)))   # [dst, chunk*128+edge%128]
        # local x transposed (for L1 local e_dst matmuls)
        xl = np.zeros((NPAD, 64), np.float32)
        xl[:NPC, :50] = x[c * NPC:(c + 1) * NPC]
        xlocs.append(np.ascontiguousarray(xl.T).astype(F16))

    # weights (shared)
    def w_aug(W, a_s, a_d, fin_pad, prev_hc=None):
        W = np.asarray(W, np.float32)
        H, C = a_s.shape
        F = W.shape[1]
        if prev_hc is not None:
            Hp, Cp = prev_hc
            perm = (np.arange(Cp)[:, None] + np.arange(Hp)[None, :] * Cp).reshape(-1)
            W = W[:, perm]
        Wp = W.reshape(H, C, F)
        Wi = np.transpose(Wp, (2, 1, 0)).reshape(F, C * H)
        es = np.einsum("hcf,hc->fh", Wp, np.asarray(a_s, np.float32))
        ed = np.einsum("hcf,hc->fh", Wp, np.asarray(a_d, np.float32))
        out = np.concatenate([Wi, es, ed], 1)
        return np.concatenate(
            [out, np.zeros((fin_pad - F, out.shape[1]), np.float32)], 0
        ).astype(F16)

    w1 = w_aug(inputs["W1"], np.asarray(inputs["as1"]), np.asarray(inputs["ad1"]), 64)
    w2 = w_aug(inputs["W2"], np.asarray(inputs["as2"]), np.asarray(inputs["ad2"]), 1024,
               prev_hc=(4, 256))
    w3 = w_aug(inputs["W3"], np.asarray(inputs["as3"]), np.asarray(inputs["ad3"]), 1024,
               prev_hc=(4, 256))

    # global x table, transposed: col j = x[node with TROW == j], pad cols 0
    xt = np.zeros((R, 64), np.float32)
    g = np.arange(N)
    xt[_trow(g), :50] = x
    x1T = np.ascontiguousarray(xt.T).astype(F16)

    shared = {"x1T": x1T, "w1": w1, "w2": w2, "w3": w3}
    percore = [
        {"gidx": gidx[c], "gidxA": gidxA[c], "st": sts[c], "st2": st2s[c],
         "x1Tloc": xlocs[c], "cnts": cnts[c]}
        for c in range(NCORES)
    ]
    return K_T, shared, percore


_CACHE = {}


def build_program(K_T):
    import concourse.bacc as bacc
    import concourse.mybir as mybir
    import concourse.tile as tile

    dt = mybir.dt
    AF = mybir.ActivationFunctionType
    AL = mybir.AluOpType
    NCH = T * K_T

    nc = bacc.Bacc("TRN2", target_bir_lowering=False, debug=False, num_devices=NCORES)

    def register_const(val):
        t = nc.alloc_sbuf_tensor(f"constx-{val}", [128, 1], dt.float32)
        nc.gpsimd.memset(t.ap(), val)
        nc.const_aps.aps[(dt.float32, val)] = t.ap()

    for s in SHIFTS:
        if (dt.float32, -s) not in nc.const_aps.aps:
            register_const(-s)
    nc.all_engine_barrier()

    x1T = nc.dram_tensor("x1T", [64, R], dt.float16, kind="ExternalInput")
    w1 = nc.dram_tensor("w1", [64, 1032], dt.float16, kind="ExternalInput")
    w2 = nc.dram_tensor("w2", [1024, 1032], dt.float16, kind="ExternalInput")
    w3 = nc.dram_tensor("w3", [1024, 738], dt.float16, kind="ExternalInput")
    gidx = nc.dram_tensor("gidx", [128, NCH * 8], dt.int16, kind="ExternalInput")
    gidxA = nc.dram_tensor("gidxA", [128, 2 * K_T * 8], dt.int16, kind="ExternalInput")
    cntsT = nc.dram_tensor("cnts", [128, T], dt.int32, kind="ExternalInput")
    st = nc.dram_tensor("st", [128, NCH * 128], dt.float16, kind="ExternalInput")
    st2 = nc.dram_tensor("st2", [128, NCH * 128], dt.float16, kind="ExternalInput")
    x1Tloc = nc.dram_tensor("x1Tloc", [64, NPAD], dt.float16, kind="ExternalInput")
    out = nc.dram_tensor("out", [NPAD, 121], dt.float32, kind="ExternalOutput")

    tableA = nc.dram_tensor("tableA", [R, WT12], dt.float16)
    tableB = nc.dram_tensor("tableB", [R, WT12], dt.float16, addr_space="Shared")
    table3 = nc.dram_tensor("table3", [R, WT3], dt.float16, addr_space="Shared")
    bounceB = nc.dram_tensor("bounceB", [NPAD, WT12], dt.float16)
    bounce3 = nc.dram_tensor("bounce3", [NPAD, WT3], dt.float16)

    RG = [list(range(NCORES))]

    with tile.TileContext(nc) as tc:
        from concourse.masks import make_identity

        with (
            tc.tile_pool(name="per", bufs=1) as per,
            tc.tile_pool(name="gsp", bufs=3) as gsp,
            tc.tile_pool(name="dp", bufs=2) as dp,
            tc.tile_pool(name="sp", bufs=2) as sp,
            tc.tile_pool(name="wp", bufs=4) as wp,
            tc.tile_pool(name="ep", bufs=2) as ep,
            tc.tile_pool(name="eps", bufs=2, space="PSUM") as eps,
            tc.tile_pool(name="aux", bufs=2, space="PSUM") as aux,
        ):
            # persistent loads
            w1s = per.tile([64, 1032], dt.float16)
            nc.sync.dma_start(w1s[:], w1[:])
            w2s = per.tile([128, 8, 1032], dt.float16)
            nc.sync.dma_start(w2s[:], w2.ap().rearrange("(a p) n -> p a n", p=128))
            w3s = per.tile([128, 8, 738], dt.float16)
            nc.sync.dma_start(w3s[:], w3.ap().rearrange("(a p) n -> p a n", p=128))
            g1i = per.tile([128, NCH * 8], dt.int16)
            nc.sync.dma_start(g1i[:], gidx[:])
            g1iA = per.tile([128, 2 * K_T * 8], dt.int16)
            nc.sync.dma_start(g1iA[:], gidxA[:])
            cnts = per.tile([128, T], dt.int32)
            nc.sync.dma_start(cnts[:], cntsT[:])
            x1s = per.tile([64, R], dt.float16)
            nc.sync.dma_start(x1s[:], x1T[:])
            x1ls = per.tile([64, NPAD], dt.float16)
            nc.sync.dma_start(x1ls[:], x1Tloc[:])
            edl1 = per.tile([128, T, 8], dt.float16)
            edlB = per.tile([128, T, 8], dt.float16)
            edl3 = per.tile([128, T, 8], dt.float16)
            idf16 = per.tile([128, 128], dt.float16)
            make_identity(nc, idf16[:])
            xTs = per.tile([128, 8, NPAD], dt.float16)
            xres = per.tile([128, T, 1024], dt.float16)
            # two manually-alternated gather buffers (byte capacity for the
            # widest row layout; L3 uses a contiguous prefix view)
            gbufs = [
                per.tile([128, K_T * WT12], dt.float16, name=f"gbuf{i}")
                for i in range(2)
            ]
            cnt_reg = nc.gpsimd.alloc_register("gcnt")

            # ---------- dense helper: one output m-tile ----------
            def dense_tile(lhsT_fn, w_sb, nk, widths, copy_fn, use_eps=False):
                for si, (o, wd) in enumerate(widths):
                    if use_eps:
                        ps = eps.tile(
                            [128, 512 if wd > 64 else 128], dt.float32,
                            tag="ABC"[si], name=f"dps{si}",
                        )
                    else:
                        ps = aux.tile([128, 512], dt.float32, tag="aux", name="psden")
                    for k in range(nk):
                        nc.tensor.matmul(
                            ps[:, :wd], lhsT_fn(k), w_sb(k, o, wd),
                            start=(k == 0), stop=(k == nk - 1),
                        )
                    copy_fn(si, o, wd, ps)

            # copy plans: write psum slice -> packed fp8/fp16 table row tile
            def copy12(tabst, edst_ap):
                # widths (0,512),(512,512),(1024,8); h fp8 at fp16 cols
                # [0,512), e_src fp16 at [512,516)
                def fn(si, o, wd, ps):
                    if si == 0:
                        nc.scalar.copy(
                            tabst[:, 0:256].bitcast(dt.float8e4), ps[:, :512]
                        )
                    elif si == 1:
                        nc.vector.tensor_copy(
                            tabst[:, 256:512].bitcast(dt.float8e4), ps[:, :512]
                        )
                    else:
                        nc.vector.tensor_copy(tabst[:, 512:516], ps[:, 0:4])
                        nc.vector.tensor_copy(edst_ap, ps[:, 4:8])
                return fn

            def copy3(tabst, edst_ap):
                # widths (0,512),(512,226); h fp8 at fp16 cols [0,363),
                # e_src fp16 at [364,370)
                def fn(si, o, wd, ps):
                    if si == 0:
                        nc.scalar.copy(
                            tabst[:, 0:256].bitcast(dt.float8e4), ps[:, :512]
                        )
                    else:
                        nc.vector.tensor_copy(
                            tabst[:, 256:363].bitcast(dt.float8e4), ps[:, 0:214]
                        )
                        nc.vector.tensor_copy(tabst[:, 364:370], ps[:, 214:220])
                        nc.vector.tensor_copy(edst_ap, ps[:, 220:226])
                return fn

            # ---------- L1 dense: all R rows, replicated ----------
            for m in range(R // P):
                tabst = ep.tile([128, WT12], dt.float16, tag="tabst", bufs=3)
                edst_st = ep.tile([128, 8], dt.float16, tag="edstst")
                dense_tile(
                    lambda k, m=m: x1s[:, m * P:(m + 1) * P],
                    lambda k, o, wd: w1s[:, o:o + wd],
                    1,
                    [(0, 512), (512, 512), (1024, 8)],
                    copy12(tabst, edst_st[:, 0:4]), use_eps=True,
                )
                nc.sync.dma_start(tableA[m * P:(m + 1) * P, 0:516], tabst[:, 0:516])

            # L1 local e_dst (tiny matmuls from local x)
            for t in range(T):
                pse = aux.tile([128, 512], dt.float32, tag="aux", name="pse")
                nc.tensor.matmul(
                    pse[:, :8], x1ls[:, t * P:(t + 1) * P], w1s[:, 1024:1032],
                    start=True, stop=True,
                )
                nc.scalar.copy(edl1[:, t, 0:4], pse[:, 4:8])

            # ---------- edge phase ----------
            def edge_phase(lidx, table, edl, WTAB, ESOFF, H, C, shift, epi_fn):
                DO = H * C
                for t in range(T):
                    g = gbufs[t % 2][:, 0:K_T * WTAB].rearrange(
                        "p (k w) -> p k w", w=WTAB
                    )
                    off8 = t * K_T * 8
                    if t < 2:
                        # full gather (pad idx = row 0): keeps the two g
                        # buffers free of cross-layer stale bytes, which the
                        # fp16 logit view could read as NaN/Inf.
                        nc.gpsimd.dma_gather(
                            g, table.ap(), g1iA[:, t * K_T * 8:(t + 1) * K_T * 8],
                            num_idxs=K_T * 128, num_idxs_reg=K_T * 128,
                            elem_size=WTAB, single_packet=False,
                        )
                    else:
                        nc.gpsimd.reg_load(cnt_reg, cnts[0:1, t:t + 1])
                        nc.gpsimd.dma_gather(
                            g, table.ap(), g1i[:, off8:off8 + K_T * 8],
                            num_idxs=K_T * 128, num_idxs_reg=cnt_reg,
                            elem_size=WTAB, single_packet=False,
                        )
                    s2 = dp.tile([128, K_T * 128], dt.float16, tag="s2")
                    nc.sync.dma_start(
                        s2[:], st2[:, t * K_T * 128:(t + 1) * K_T * 128]
                    )
                    s = sp.tile([128, K_T * 128], dt.float16, tag="s")
                    nc.sync.dma_start(
                        s[:], st[:, t * K_T * 128:(t + 1) * K_T * 128]
                    )
                    psA = eps.tile([128, 512], dt.float32, tag="A")
                    psB = eps.tile([128, 512], dt.float32, tag="B")
                    # psD (per-edge e_dst, cols 0:K_T*H) and psC (denominator,
                    # cols 112:112+H) share one PSUM bank.
                    psCD = eps.tile([128, 128], dt.float32, tag="C", name="psCD")
                    psC = psCD[:, 112:112 + H]
                    for k in range(K_T):
                        nc.tensor.matmul(
                            psCD[:, k * H:(k + 1) * H],
                            s2[:, k * 128:(k + 1) * 128],
                            edl[:, t, 0:H],
                            start=True, stop=True,
                        )
                    # es = e_src(gathered) + e_dst; leaky; exp
                    glog = g[:, :, ESOFF:ESOFF + H]
                    es = wp.tile([128, K_T, H], dt.float32, tag="es")
                    nc.vector.tensor_tensor(
                        es[:], glog,
                        psCD[:, 0:K_T * H].rearrange("p (b h) -> p b h", h=H),
                        op=AL.add,
                    )
                    nc.vector.scalar_tensor_tensor(
                        es[:], es[:], 0.2, es[:], op0=AL.mult, op1=AL.max
                    )
                    esx = wp.tile([128, K_T, H], dt.float16, tag="esx")
                    nc.scalar.activation(esx[:], es[:], AF.Exp, bias=-shift)
                    bw = min(512, DO - 512)
                    for k in range(K_T):
                        gs = gsp.tile([128, 1024], dt.float16, tag="gs")
                        nc.vector.tensor_tensor(
                            gs[:, 0:DO].rearrange("p (c h) -> p c h", h=H),
                            g[:, k, 0:DO // 2].bitcast(dt.float8e4).rearrange(
                                "p (c h) -> p c h", h=H
                            ),
                            esx[:, k].rearrange("p (o h) -> p o h", o=1)
                            .to_broadcast([128, C, H]),
                            op=AL.mult,
                        )
                        fl, ll = k == 0, k == K_T - 1
                        sT = s[:, k * 128:(k + 1) * 128]
                        nc.tensor.matmul(
                            psA[:], sT, gs[:, 0:512], start=fl, stop=ll
                        )
                        nc.tensor.matmul(
                            psB[:, :bw], sT, gs[:, 512:512 + bw],
                            start=fl, stop=ll,
                        )
                        nc.tensor.matmul(
                            psC, sT, esx[:, k], start=fl, stop=ll
                        )
                    epi_fn(t, psA, psB, psC)

            # ---------- epilogues ----------
            def normalize12(t, psA, psB, psC, H, C):
                dn = wp.tile([128, H], dt.float32, tag="dn")
                nc.vector.tensor_scalar_max(dn[:], psC, 1e-16)
                r = wp.tile([128, H], dt.float32, tag="r")
                nc.vector.reciprocal(r[:], dn[:])
                xt = ep.tile([128, 1024], dt.float16, tag="xt")
                rb = r[:].rearrange("p (o h) -> p o h", o=1).to_broadcast([128, 128, H])
                for half, ps in ((0, psA), (1, psB)):
                    nc.vector.tensor_tensor(
                        xt[:, half * 512:(half + 1) * 512].rearrange(
                            "p (c h) -> p c h", h=H
                        ),
                        ps[:].rearrange("p (c h) -> p c h", h=H),
                        rb, op=AL.mult,
                    )
                return xt

            def elu_into(xt, dest):
                neg = ep.tile([128, 1024], dt.float16, tag="neg")
                nc.vector.tensor_scalar_min(neg[:], xt[:], 0.0)
                en = ep.tile([128, 1024], dt.float16, tag="en")
                nc.scalar.activation(en[:], neg[:], AF.Exp)
                a = ep.tile([128, 1024], dt.float16, tag="a")
                nc.vector.tensor_sub(a[:], xt[:], neg[:])
                nc.vector.scalar_tensor_tensor(
                    dest, a[:], -1.0, en[:], op0=AL.add, op1=AL.add
                )

            def transpose_dense(t, xsrc, w_sb, widths, bounce, copy_fn, tabcols,
                                tableN):
                # PE-transpose x tile into xTs: all 8 blocks into one 1-bank
                # psum tile, drained by a single strided copy.
                tp = aux.tile([128, 8, 128], dt.float16, tag="aux", name="tpk")
                for fb in range(8):
                    nc.tensor.transpose(
                        tp[:, fb, :], xsrc[:, fb * 128:(fb + 1) * 128], idf16[:]
                    )
                nc.scalar.copy(xTs[:, :, t * P:(t + 1) * P], tp[:])
                dense_tile(
                    lambda k, t=t: xTs[:, k, t * P:(t + 1) * P],
                    lambda k, o, wd: w_sb[:, k, o:o + wd],
                    8, widths, copy_fn,
                )
                nc.sync.dma_start(
                    bounce[t * P:(t + 1) * P, 0:tabcols], tabst_cur[0][:, 0:tabcols]
                )
                if t % 2 == 1:
                    nc.gpsimd.collective_compute(
                        "AllGather", mybir.AluOpType.bypass,
                        ins=[bounce[(t - 1) * P:(t + 1) * P, :].opt()],
                        outs=[tableN[(t - 1) * 1024:(t + 1) * 1024, :].opt()],
                        replica_groups=RG,
                    )

            tabst_cur = [None]

            # L1 edge: epilogue computes x2 (elu), stores to xres, builds L2 table
            def epi1(t, psA, psB, psC):
                xt = normalize12(t, psA, psB, psC, 4, 256)
                elu_into(xt, xres[:, t, :])
                tabst = ep.tile([128, WT12], dt.float16, tag="tabst", bufs=3)
                edst_st = ep.tile([128, 8], dt.float16, tag="edstst")
                tabst_cur[0] = tabst
                transpose_dense(
                    t, xres[:, t, :], w2s,
                    [(0, 512), (512, 512), (1024, 8)],
                    bounceB, copy12(tabst, edlB[:, t, 0:4]), 516, tableB,
                )

            edge_phase(0, tableA, edl1, WT12, ESOFF12, 4, 256, SHIFTS[0], epi1)

            # L2 edge: epilogue x3 = elu(norm + x2), builds L3 table
            def epi2(t, psA, psB, psC):
                xt = normalize12(t, psA, psB, psC, 4, 256)
                nc.vector.tensor_add(xt[:], xt[:], xres[:, t, :])
                x3 = ep.tile([128, 1024], dt.float16, tag="x3")
                elu_into(xt, x3[:])
                tabst = ep.tile([128, WT3], dt.float16, tag="tabst3", bufs=3)
                tabst_cur[0] = tabst
                transpose_dense(
                    t, x3[:], w3s,
                    [(0, 512), (512, 226)],
                    bounce3, copy3(tabst, edl3[:, t, 0:6]), 370, table3,
                )

            edge_phase(1, tableB, edlB, WT12, ESOFF12, 4, 256, SHIFTS[1], epi2)

            # L3 edge: final epilogue
            def epi3(t, psA, psB, psC):
                t726 = ep.tile([128, 726], dt.float32, tag="t726")
                nc.vector.tensor_copy(t726[:, 0:512], psA[:])
                nc.vector.tensor_copy(t726[:, 512:726], psB[:, 0:214])
                dn = wp.tile([128, 6], dt.float32, tag="dn")
                nc.vector.tensor_scalar_max(dn[:], psC, 1e-16)
                r = wp.tile([128, 6], dt.float32, tag="r")
                nc.vector.reciprocal(r[:], dn[:])
                r6 = wp.tile([128, 6], dt.float32, tag="r6")
                nc.vector.tensor_scalar_mul(r6[:], r[:], 1.0 / 6.0)
                tmp = ep.tile([128, 726], dt.float32, tag="tmp726")
                nc.vector.tensor_tensor(
                    tmp[:].rearrange("p (c h) -> p c h", h=6),
                    t726[:].rearrange("p (c h) -> p c h", h=6),
                    r6[:].rearrange("p (o h) -> p o h", o=1).to_broadcast(
                        [128, 121, 6]
                    ),
                    op=AL.mult,
                )
                o121 = ep.tile([128, 121], dt.float32, tag="o121")
                nc.vector.reduce_sum(
                    o121[:], tmp[:].rearrange("p (c h) -> p c h", h=6),
                    mybir.AxisListType.X,
                )
                nc.scalar.activation(o121[:], o121[:], AF.Sigmoid)
                nc.sync.dma_start(out[t * P:(t + 1) * P, :], o121[:])

            edge_phase(2, table3, edl3, WT3, ESOFF3, 6, 121, SHIFTS[2], epi3)

    nc.compile()
    return nc


def run(inputs, trace=False, tmpdir=None):
    from concourse.bass_utils import run_bass_kernel_spmd

    K_T, shared, percore = preprocess(inputs)
    key = K_T
    if key not in _CACHE:
        _CACHE[key] = build_program(K_T)
    nc = _CACHE[key]
    in_maps = [{**shared, **percore[c]} for c in range(NCORES)]
    if trace:
        import types

        try:
            import antenv.axon_hooks  # noqa: F401
        except ImportError:
            from trn_agent_boot.trn_boot import _ntff_profile_via_ctypes

            m = types.ModuleType("antenv.axon_hooks")
            hook = _ntff_profile_via_ctypes("/opt/axon/libaxon_pjrt.so")
            m.get_axon_ntff_profile_hook = lambda: hook
            sys.modules["antenv.axon_hooks"] = m
    try:
        res = run_bass_kernel_spmd(
            nc, in_maps, list(range(NCORES)), trace=trace, tmpdir=tmpdir
        )
    except Exception:
        # A crashed earlier run can leave the device wedged; one retry
        # normally clears it (nrt reopen).
        import time

        time.sleep(2)
        res = run_bass_kernel_spmd(
            nc, in_maps, list(range(NCORES)), trace=trace, tmpdir=tmpdir
        )
    outs = []
    for c in range(NCORES):
        outs.append(res.results[c]["out"][:NPC])
    full = np.concatenate(outs, 0).astype(np.float32)
    return full, res


def kernel(**inputs):
    full, _ = run(inputs)
    return full
